# revision 25
# baseline (speedup 1.0000x reference)
"""BinaryLinear kernel for 8 Trainium2 NeuronCores.

y = x @ (scale * sign(weight))^T,  x:[8192,4096] f32, weight:[4096,4096] f32.

Strategy: data-parallel token split (1024 tokens/core), weight replicated.
All 32 K-subtiles (128 each) run as fp8e4 (e4m3) matmuls in DoubleRow perf
mode. Casting, binarization and the 0.5 scale are folded into host-side
packing, so the device program is pure DMA + matmul + PSUM drain.

Device schedule — overhead trimming around the ~221us PE-streaming
floor of 1024 DoubleRow matmuls x 216ns (the fp8 peak: 512 moving
columns + ~6 cycles at 2.4GHz each; measured steady-state spacing is
exactly 216ns, so the middle of the kernel is roofline-pinned):
  * 34 zero-input warmup matmuls (N=128, no data deps) run right after
    the preamble barrier so the PE HAM activity window fills while the
    first input DMAs are in flight. Any PE idle gap degrades the HAM
    busy window and delays the cold(1.2GHz)->warm(2.4GHz) flip, so the
    warmups are sized to bridge all the way to data arrival (~11.5us).
  * x is packed [kp, P, t, 2, 128]; kp=0 lands as a t=0 piece plus a
    t=1..7 piece, both issued BEFORE the first w chunk, so every kp=0
    LDWEIGHTS is unblocked when w(0,0) arrives and the matmul stream
    pipelines immediately (blocked LDWs run matmuls in isolated,
    unpipelined ~650ns mode).
  * w is resident: 16 tiles [P, 2, 4096] (one per k-pair), written by
    per-(o,kp) slice DMAs in consumption order; no buffer recycling.
    Phase-A w chunks ride the scalar engine's hardware DGE queue,
    parallel to x on the sync queue.
  * Stores: slabs 0..6 batch 8 PSUM drains into one [P, 8, 512] SBUF
    stage (drain copies alternate Vector/Scalar, which access PSUM in
    parallel and keep the tile scheduler's simulated engine clocks from
    pinning the copies' baked sem waits late) and store with a single
    DMA. The last slab stores per t-tile from fresh small stage tiles
    (a shared stage would chain copy(t+1) behind store(t) via WAR), on
    alternating sync/scalar queues, and its final t runs as 2x256-col
    accumulation groups so the last store overlaps the tail matmuls.
  * gpsimd SWDGE is unused; y is written p-major and transposed on host.

Accuracy: e4m3 quantization of x dominates the error. The host pack uses
discrepancy-steered rounding (error diffusion against the actual sign
matrix + coordinate-descent + bilinear max-shaving sweeps), cutting the
max error to 1.56e-2 vs the 2e-2 gate. Device accumulation order is
identical to the baseline kernel, so results are bit-identical to it.
"""

import functools

import numpy as np
import ml_dtypes

TOKENS = 8192
IN_F = 4096
OUT_F = 4096
N_CORES = 8
TS = TOKENS // N_CORES  # tokens per core

P = 128        # partitions / contraction tile
N_TILE = 512   # matmul moving free dim (one PSUM bank of f32)
K_TILES = IN_F // P          # 32
T_TILES = TS // P            # 8
O_TILES = OUT_F // N_TILE    # 8
PSUM_BUFS = 8

NKP8 = K_TILES // 2          # DoubleRow k-pairs (16)
K8 = IN_F

N_WARMUP_MM = 34             # zero-input HAM warmup matmuls (N=128 each)

STEER_CAND_SIDE = 4          # e4m3 grid candidates per side (8 total)
STEER_SWEEPS = 3             # L2 sweeps (incl. first greedy pass)
STEER_BILIN_SWEEPS = 1       # bilinear max-shaving sweeps
STEER_LAM = 64.0

FP8_NP = ml_dtypes.float8_e4m3  # TRN fp8e4 (max 240, RNE)
BLK = 128                       # steering block size


def _build_program():
    import concourse.bacc as bacc
    import concourse.mybir as mybir
    import concourse.tile as tile

    fp32 = mybir.dt.float32
    fp8 = mybir.dt.float8e4
    DR = mybir.MatmulPerfMode.DoubleRow

    nc = bacc.Bacc(
        "TRN2",
        target_bir_lowering=False,
        debug=False,
        num_devices=N_CORES,
    )
    # x: [kp, p, t, 2, 128] so per-(kp,t) pieces are contiguous 32KiB
    x8_d = nc.dram_tensor(
        "x8", [NKP8, P, T_TILES, 2, P], fp8, kind="ExternalInput"
    ).ap()
    w8_d = nc.dram_tensor(
        "w8", [NKP8, O_TILES, P, 2, N_TILE], fp8, kind="ExternalInput"
    ).ap()
    # y written p-major: [p, t, o, c]; host transposes back to [t*128+p, o*512+c]
    y_d = nc.dram_tensor(
        "y", [P, T_TILES, O_TILES, N_TILE], fp32, kind="ExternalOutput"
    ).ap()

    with tile.TileContext(nc) as tc:
        with (
            tc.tile_pool(name="dummy", bufs=1) as dummy_pool,
            tc.tile_pool(name="xres", bufs=NKP8) as x_pool,
            tc.tile_pool(name="wres", bufs=NKP8) as w_pool,
            tc.tile_pool(name="ostage", bufs=1) as ostage_pool,
            tc.tile_pool(name="otail", bufs=6) as otail_pool,
            tc.tile_pool(name="psum", bufs=PSUM_BUFS, space="PSUM") as psum_pool,
        ):
            # --- HAM warmup: zero matmuls with no data dependencies ---
            dz = dummy_pool.tile([P, 2, P], fp8, tag="dz", name="dz")
            nc.gpsimd.memset(dz[:], 0)
            ps_warm = psum_pool.tile([P, N_TILE], fp32, tag="ps", name="psw")
            for _ in range(N_WARMUP_MM):
                nc.tensor.matmul(
                    ps_warm[:, 0:P],
                    dz[:],
                    dz[:],
                    start=True,
                    stop=True,
                    perf_mode=DR,
                    skip_group_check=True,
                )

            x8s = []   # resident fp8 x tiles, [P, t, 2, 128] each (k-pair)
            w8s = []   # resident fp8 w tiles, [P, 2, OUT_F] each (k-pair)
            for kp in range(NKP8):
                x8s.append(
                    x_pool.tile([P, T_TILES, 2, P], fp8, tag="x8", name="x8")
                )
                w8s.append(
                    w_pool.tile([P, 2, OUT_F], fp8, tag="w8", name="w8")
                )

            def load_w(kp, o, eng=None):
                (eng or nc.sync).dma_start(
                    w8s[kp][:, :, o * N_TILE : (o + 1) * N_TILE], w8_d[kp, o]
                )

            # Phase A: kp=0's x lands as a small t=0 piece plus one
            # t=1..7 piece, both BEFORE the first w chunk, so when w(0,0)
            # arrives every kp=0 LDWEIGHTS is already unblocked and the
            # MM stream pipelines with no PE gap (a gap degrades the HAM
            # busy window and runs matmuls in isolated, unpipelined mode).
            # x rides the sync hardware DGE queue; the first w chunks ride
            # the scalar engine's own hardware queue so the two streams'
            # descriptors are processed in parallel and the first matmul's
            # operands land ~1us sooner.
            nc.sync.dma_start(x8s[0][:, 0], x8_d[0, :, 0])
            nc.sync.dma_start(x8s[0][:, 1:T_TILES], x8_d[0, :, 1:T_TILES])
            load_w(0, 0, nc.scalar)
            for kp in range(1, NKP8):
                nc.sync.dma_start(x8s[kp][:], x8_d[kp])
                load_w(kp, 0, nc.scalar)

            def stat(kp, t):
                return x8s[kp][:, t]

            def mov(kp, o, piece=None):
                if piece is None:
                    return w8s[kp][:, :, o * N_TILE : (o + 1) * N_TILE]
                lo = o * N_TILE + piece * (N_TILE // 2)
                return w8s[kp][:, :, lo : lo + N_TILE // 2]

            for o in range(O_TILES):
                if o > 0:
                    for kp in range(NKP8):
                        load_w(kp, o)
                if o < O_TILES - 1:
                    ot = ostage_pool.tile(
                        [P, T_TILES, N_TILE], fp32, tag="ot", name="ot"
                    )
                    ps = [
                        psum_pool.tile([P, N_TILE], fp32, tag="ps", name="ps")
                        for _ in range(T_TILES)
                    ]
                    for kp in range(NKP8):
                        for t in range(T_TILES):
                            nc.tensor.matmul(
                                ps[t][:],
                                stat(kp, t),
                                mov(kp, o),
                                start=(kp == 0),
                                stop=(kp == NKP8 - 1),
                                perf_mode=DR,
                                skip_group_check=True,
                            )
                    # Drains alternate Vector/Scalar: the two engines hit
                    # different PSUM banks in parallel, and splitting the
                    # copy load keeps the tile scheduler's simulated
                    # engine clocks (which set the baked sem waits) from
                    # backlogging one engine and pinning copies late.
                    for t in range(T_TILES):
                        if t % 2 == 0:
                            nc.vector.tensor_copy(ot[:, t], ps[t][:])
                        else:
                            nc.scalar.copy(ot[:, t], ps[t][:])
                    # one batched store per slab
                    nc.sync.dma_start(y_d[:, :, o], ot[:])
                else:
                    # Last slab t-outer so final drains stagger; fresh
                    # staging tiles per store so copies never chain behind
                    # a previous store (shared-tile WAR). The final t runs
                    # as 2 half-width accumulation groups in separate PSUM
                    # tiles so the first half's store overlaps the second
                    # half's matmuls.
                    for t in range(T_TILES - 1):
                        pst = psum_pool.tile(
                            [P, N_TILE], fp32, tag="ps", name="ps"
                        )
                        for kp in range(NKP8):
                            nc.tensor.matmul(
                                pst[:],
                                stat(kp, t),
                                mov(kp, o),
                                start=(kp == 0),
                                stop=(kp == NKP8 - 1),
                                perf_mode=DR,
                                skip_group_check=True,
                            )
                        ott = otail_pool.tile(
                            [P, N_TILE], fp32, tag="ott", name="ott"
                        )
                        if t % 2 == 0:
                            nc.vector.tensor_copy(ott[:], pst[:])
                            nc.sync.dma_start(y_d[:, t, o], ott[:])
                        else:
                            nc.scalar.copy(ott[:], pst[:])
                            nc.scalar.dma_start(y_d[:, t, o], ott[:])
                    t = T_TILES - 1
                    for piece in range(2):
                        psh = psum_pool.tile(
                            [P, 256], fp32, tag="ps", name="psh"
                        )
                        for kp in range(NKP8):
                            nc.tensor.matmul(
                                psh[:],
                                stat(kp, t),
                                mov(kp, o, piece),
                                start=(kp == 0),
                                stop=(kp == NKP8 - 1),
                                perf_mode=DR,
                                skip_group_check=True,
                            )
                        oth = otail_pool.tile(
                            [P, 256], fp32, tag="ott", name="oth"
                        )
                        if piece == 0:
                            nc.scalar.copy(oth[:], psh[:])
                            nc.scalar.dma_start(
                                y_d[:, t, o, 0:256], oth[:]
                            )
                        else:
                            # one copy, then two half-stores from disjoint
                            # slices issued in parallel on both hardware
                            # queues: the final transfer + its semaphore
                            # aggregation (the critical chain into the end
                            # barrier) runs at half length. Two readers of
                            # one staged tile don't WAR-chain (a second
                            # WRITE after a store's read would).
                            nc.vector.tensor_copy(oth[:], psh[:])
                            nc.sync.dma_start(
                                y_d[:, t, o, 256:384], oth[:, 0:128]
                            )
                            nc.scalar.dma_start(
                                y_d[:, t, o, 384:512], oth[:, 128:256]
                            )

    nc.compile()
    return nc


# ---------------------------------------------------------------------------
# Host-side packing: discrepancy-steered e4m3 rounding for the fp8 K range.
# ---------------------------------------------------------------------------


def _candidates_np(xs_blk):
    """xs_blk [T, B] f32 -> candidate errors [2*SIDE, T, B] on the e4m3 grid."""
    F8 = FP8_NP
    g = xs_blk.astype(F8)
    gf = g.astype(np.float32)
    up = np.nextafter(g, F8(240.0)).astype(np.float32)
    dn = np.nextafter(g, F8(-240.0)).astype(np.float32)
    lo = np.where(gf <= xs_blk, gf, dn)
    hi = np.where(gf >= xs_blk, gf, up)
    out = []
    cl = lo
    for _ in range(STEER_CAND_SIDE):
        out.append(cl)
        cl = np.nextafter(cl.astype(F8), F8(-240.0)).astype(np.float32)
    ch = hi
    for _ in range(STEER_CAND_SIDE):
        out.append(ch)
        ch = np.nextafter(ch.astype(F8), F8(240.0)).astype(np.float32)
    return np.stack(out, axis=0) - xs_blk[None]


@functools.lru_cache(maxsize=1)
def _steer_jit_fns():
    import jax
    import jax.numpy as jnp

    @functools.partial(jax.jit, donate_argnums=(0,), static_argnames=("nout",))
    def block_step(Pimg, S_b, ce_all, nout):
        G = S_b.T @ S_b
        C = Pimg @ S_b

        def step(C, j):
            c = C[:, j]
            ce = ce_all[:, :, j]
            cost = 2.0 * ce * c[None, :] + ce * ce * jnp.float32(nout)
            idx = jnp.argmin(cost, axis=0)
            e = jnp.take_along_axis(ce, idx[None, :], axis=0)[0]
            return C + jnp.outer(e, G[j]), e

        C, E = jax.lax.scan(step, C, jnp.arange(BLK))
        return Pimg + E.T @ S_b.T, E.T

    @functools.partial(jax.jit, donate_argnums=(0,), static_argnames=("nout",))
    def block_resweep(Pimg, S_b, ce_all, E_old, nout):
        G = S_b.T @ S_b
        C = Pimg @ S_b

        def step(C, j):
            e_old = E_old[:, j]
            c = C[:, j] - e_old * jnp.float32(nout)
            ce = ce_all[:, :, j]
            cost = 2.0 * ce * c[None, :] + ce * ce * jnp.float32(nout)
            idx = jnp.argmin(cost, axis=0)
            e = jnp.take_along_axis(ce, idx[None, :], axis=0)[0]
            return C + jnp.outer(e - e_old, G[j]), e

        C, E = jax.lax.scan(step, C, jnp.arange(BLK))
        return Pimg + (E.T - E_old) @ S_b.T, E.T

    @functools.partial(jax.jit, donate_argnums=(0,), static_argnames=("nout",))
    def block_bilin(Pimg, S_b, ce_all, E_old, u, v, nout):
        # weighted objective: sum_o (1 + u_t v_o) p_to^2
        G = S_b.T @ S_b
        Gv = (S_b * v[:, None]).T @ S_b
        sv = jnp.sum(v)
        C = Pimg @ S_b
        Cv = (Pimg * v[None, :]) @ S_b

        def step(carry, j):
            C, Cv = carry
            e_old = E_old[:, j]
            c = C[:, j] - e_old * jnp.float32(nout)
            cv = Cv[:, j] - e_old * sv
            ce = ce_all[:, :, j]
            cost = 2.0 * ce * (c + u * cv)[None, :] + ce * ce * (
                jnp.float32(nout) + u * sv
            )[None, :]
            idx = jnp.argmin(cost, axis=0)
            e = jnp.take_along_axis(ce, idx[None, :], axis=0)[0]
            d = e - e_old
            return (C + jnp.outer(d, G[j]), Cv + jnp.outer(d, Gv[j])), e

        (C, Cv), E = jax.lax.scan(step, (C, Cv), jnp.arange(BLK))
        return Pimg + (E.T - E_old) @ S_b.T, E.T

    return block_step, block_resweep, block_bilin


def _steer_quantize(xs8, s8):
    """xs8: [T, K8] f32 (pre-scaled x columns for the fp8 range).
    s8: [O, K8] f32 signs (+-1). Returns [T, K8] f32 on the e4m3 grid.
    """
    import jax
    import jax.numpy as jnp

    block_step, block_resweep, block_bilin = _steer_jit_fns()
    cpu = jax.local_devices(backend="cpu")[0]
    T, K8_ = xs8.shape
    O = s8.shape[0]
    nblk = K8_ // BLK
    assert nblk * BLK == K8_

    sblocks = [
        np.ascontiguousarray(s8[:, b * BLK : (b + 1) * BLK]) for b in range(nblk)
    ]
    cands = [
        _candidates_np(xs8[:, b * BLK : (b + 1) * BLK]) for b in range(nblk)
    ]
    with jax.default_device(cpu):
        Pimg = jnp.zeros((T, O), dtype=np.float32)
        E = [None] * nblk
        for b in range(nblk):
            Pimg, E[b] = block_step(Pimg, sblocks[b], cands[b], O)
        for _ in range(STEER_SWEEPS - 1):
            for b in range(nblk):
                Pimg, E[b] = block_resweep(Pimg, sblocks[b], cands[b], E[b], O)
        for _ in range(STEER_BILIN_SWEEPS):
            Pn = np.abs(np.asarray(Pimg))
            tmax = Pn.max(axis=1)
            omax = Pn.max(axis=0)
            u = jnp.asarray(
                np.float32(STEER_LAM) * (tmax / tmax.max()) ** 4
            )
            v = jnp.asarray((omax / omax.max()).astype(np.float32) ** 4)
            for b in range(nblk):
                Pimg, E[b] = block_bilin(
                    Pimg, sblocks[b], cands[b], E[b], u, v, O
                )
        out = np.concatenate([np.asarray(e) for e in E], axis=1) + xs8
    return out


def _pack_weights(weight):
    """sign(weight) packed for fp8 DoubleRow layout: [kp, o, P, 2, 512]."""
    s = np.where(weight >= 0, np.float32(1.0), np.float32(-1.0))
    sT = np.ascontiguousarray(s.T)  # [IN_F, OUT_F]
    w8 = (
        sT.reshape(NKP8, 2, P, O_TILES, N_TILE)
        .transpose(0, 3, 2, 1, 4)
        .astype(FP8_NP)
    )
    w8 = np.ascontiguousarray(w8)
    return s, w8


def run(x, weight, scale, trace=False, tmpdir=None):
    from concourse.bass_utils import run_bass_kernel_spmd

    x = np.asarray(x, dtype=np.float32)
    weight = np.asarray(weight, dtype=np.float32)
    sc = float(np.asarray(scale))

    assert x.shape == (TOKENS, IN_F), x.shape
    assert weight.shape == (OUT_F, IN_F), weight.shape

    nc = _build_program()

    s, w8 = _pack_weights(weight)
    xs = x * np.float32(sc)  # fold scale into x on host
    xq8 = _steer_quantize(xs, s)  # all tokens at once

    in_maps = []
    for c in range(N_CORES):
        sl = slice(c * TS, (c + 1) * TS)
        # [TS, K] -> [kp, P, t, 2, 128]
        x8 = (
            xq8[sl]
            .T.reshape(NKP8, 2, P, T_TILES, P)
            .transpose(0, 2, 3, 1, 4)
            .astype(FP8_NP)
        )
        in_maps.append({"x8": np.ascontiguousarray(x8), "w8": w8})

    res = run_bass_kernel_spmd(
        nc,
        in_maps,
        core_ids=list(range(N_CORES)),
        trace=trace,
        tmpdir=tmpdir,
    )
    # y comes back p-major [P, T_TILES, O_TILES, N_TILE]
    y = np.concatenate(
        [
            res.results[c]["y"].transpose(1, 0, 2, 3).reshape(TS, OUT_F)
            for c in range(N_CORES)
        ],
        axis=0,
    )
    return np.ascontiguousarray(y).astype(np.float32, copy=False), res


def kernel(x, weight, scale):
    y, _ = run(x, weight, scale, trace=False)
    return y
